# revision 1
# baseline (speedup 1.0000x reference)
"""GCNConv (normalize=True, self-loops) + ReLU on 8 Trainium2 NeuronCores.

Strategy (1D node partition, per sharding hint):
  - nodes sharded 8 ways; core k owns rows [k*12500, (k+1)*12500) and all
    edges whose DESTINATION is local.
  - launch A (per core): h = x_k @ W, dinv = 1/sqrt(deg), hs = h*dinv,
    also writes hs^T. deg comes from per-dest edge counts (+1 self loop).
  - host: all-gather of hs shards into one table (pure data movement).
  - launch B (per core): for each 128-dest window, gather source rows of hs
    (dma_gather, int16 indices per 32768-row bucket), build 0/1 dest
    indicator per 128-edge chunk on DVE (is_equal vs iota), and segment-sum
    via PE matmul accumulating in PSUM [64 feat x 128 dest]; finally
    (+hs_own^T) * dinv + b, relu.

Edges are bucketed by (source-bucket q, dest-window w) with a chunk schedule
S[q][w] shared across cores (max over cores) so all 8 cores run one NEFF.
"""
import sys

sys.path.insert(0, "/opt/trn_rl_repo")
import numpy as np

N = 100000
E_DEFAULT = 1600000
DIN = 256
DOUT = 64
M = 8
P = 128
BUCKET = 32768

_cache = {}


def _ceil_div(a, b):
    return (a + b - 1) // b


class GCNConfig:
    def __init__(self, n=N, din=DIN, dout=DOUT, m=M, sbw=7):
        self.n = n
        self.din = din
        self.dout = dout
        self.m = m
        self.nl = n // m
        assert self.nl * m == n
        self.nw = _ceil_div(self.nl, P)
        self.nlp = self.nw * P
        self.nq = _ceil_div(m * self.nlp, BUCKET)
        self.sbw = sbw
        self.sbs = [range(i, min(i + sbw, self.nw)) for i in range(0, self.nw, sbw)]


def _preprocess(cfg, edge_index):
    """Partition + bucket edges; build per-core gather streams and the shared
    chunk schedule. Returns (S, Qb, C, Lq, percore_arrays)."""
    nl, nw, nlp, nq, m = cfg.nl, cfg.nw, cfg.nlp, cfg.nq, cfg.m
    ei = np.asarray(edge_index, dtype=np.int64)
    row, col = ei[0], ei[1]
    kown = col // nl
    dl = col % nl
    gsrc = (row // nl) * nlp + (row % nl)
    qb_ = gsrc // BUCKET

    cores = []
    cnts = np.zeros((m, nq, nw), np.int64)
    for k in range(m):
        sel = kown == k
        dlk = dl[sel]
        gk = gsrc[sel]
        qk = qb_[sel]
        o = np.lexsort((dlk, qk))
        dlk, gk, qk = dlk[o], gk[o], qk[o]
        wk = dlk // P
        cnts[k] = np.bincount(qk * nw + wk, minlength=nq * nw).reshape(nq, nw)
        cores.append((dlk, gk, qk, wk))

    S = _ceil_div(cnts.max(axis=0), P)  # [nq, nw] chunks per group
    Sq = S.sum(axis=1)  # chunks per stream q
    Lq = Sq * P  # idx slots per stream q
    Qb = np.concatenate([[0], np.cumsum(Sq)])  # global chunk base per q
    C = int(Qb[-1])
    chb = np.cumsum(S, axis=1) - S  # chunk base of (q,w) within stream q

    percore = []
    for k in range(m):
        dlk, gk, qk, wk = cores[k]
        nk = len(dlk)
        key = qk * nw + wk
        if nk:
            starts = np.r_[0, np.flatnonzero(np.diff(key)) + 1]
            lens = np.diff(np.r_[starts, nk])
            j = np.arange(nk) - np.repeat(starts, lens)
        else:
            j = np.zeros(0, np.int64)
        pos = chb[qk, wk] * P + j  # slot within stream q
        gpos = (Qb[qk] + chb[qk, wk]) * P + j  # global slot
        idxs = []
        for q in range(nq):
            arr = np.zeros(int(Lq[q]), np.int16)
            selq = qk == q
            arr[pos[selq]] = (gk[selq] % BUCKET).astype(np.int16)
            if Lq[q]:
                a = np.ascontiguousarray(np.tile(arr.reshape(-1, 16).T, (8, 1)))
            else:
                a = np.zeros((P, 0), np.int16)
            idxs.append(a)
        dshT = np.full(C * P, -1.0, np.float32)
        dshT[gpos] = (dlk - wk * P).astype(np.float32)
        dsh = np.ascontiguousarray(dshT.reshape(C, P).T)
        cnt2d = np.ascontiguousarray(
            np.bincount(dlk, minlength=nlp).reshape(nw, P).T
        ).astype(np.float32)
        percore.append({"idxs": idxs, "dsh": dsh, "cnt2d": cnt2d})
    return S, Qb, C, Lq, percore


def _build_launch_a(cfg):
    import concourse.mybir as mybir
    import concourse.tile as tile
    from concourse import bacc

    f32 = mybir.dt.float32
    din, dout, nw, nlp = cfg.din, cfg.dout, cfg.nw, cfg.nlp
    kc = din // P
    nc = bacc.Bacc("TRN2", target_bir_lowering=False, debug=False,
                   enable_asserts=False, num_devices=cfg.m)
    xT = nc.dram_tensor("xT", [din, nlp], f32, kind="ExternalInput")
    Wt = nc.dram_tensor("W", [din, dout], f32, kind="ExternalInput")
    cnt = nc.dram_tensor("cnt", [P, nw], f32, kind="ExternalInput")
    ident = nc.dram_tensor("ident", [P, P], f32, kind="ExternalInput")
    hs = nc.dram_tensor("hs", [nlp, dout], f32, kind="ExternalOutput")
    hsT = nc.dram_tensor("hsT", [dout, nlp], f32, kind="ExternalOutput")
    dinv = nc.dram_tensor("dinv", [P, nw], f32, kind="ExternalOutput")
    with tile.TileContext(nc) as tc:
        with tc.tile_pool(name="const", bufs=1) as cpool, \
             tc.tile_pool(name="work", bufs=4) as wpool, \
             tc.tile_pool(name="psum", bufs=4, space="PSUM") as ppool:
            xsb = cpool.tile([P, kc, nlp], f32)
            nc.sync.dma_start(out=xsb[:], in_=xT[:, :].rearrange("(c p) m -> p c m", p=P))
            wsb = cpool.tile([P, kc, dout], f32)
            nc.sync.dma_start(out=wsb[:], in_=Wt[:, :].rearrange("(c p) n -> p c n", p=P))
            idsb = cpool.tile([P, P], f32)
            nc.sync.dma_start(out=idsb[:], in_=ident[:, :])
            cntsb = cpool.tile([P, nw], f32)
            nc.sync.dma_start(out=cntsb[:], in_=cnt[:, :])
            ssb = cpool.tile([P, nw], f32)
            nc.scalar.activation(out=ssb[:], in_=cntsb[:],
                                 func=mybir.ActivationFunctionType.Sqrt, bias=1.0)
            dsb = cpool.tile([P, nw], f32)
            nc.vector.reciprocal(out=dsb[:], in_=ssb[:])
            nc.sync.dma_start(out=dinv[:, :], in_=dsb[:])
            for mm in range(nw):
                ps = ppool.tile([P, dout], f32, tag="mm")
                for c in range(kc):
                    nc.tensor.matmul(out=ps[:], lhsT=xsb[:, c, mm * P:(mm + 1) * P],
                                     rhs=wsb[:, c, :], start=(c == 0), stop=(c == kc - 1))
                hst = wpool.tile([P, dout], f32, tag="hs")
                nc.vector.tensor_scalar_mul(out=hst[:], in0=ps[:], scalar1=dsb[:, mm:mm + 1])
                nc.sync.dma_start(out=hs[mm * P:(mm + 1) * P, :], in_=hst[:])
                psT = ppool.tile([dout, P], f32, tag="tr")
                nc.tensor.transpose(out=psT[:], in_=hst[:], identity=idsb[:])
                hstT = wpool.tile([dout, P], f32, tag="hsT")
                nc.vector.tensor_copy(out=hstT[:], in_=psT[:])
                nc.sync.dma_start(out=hsT[:, mm * P:(mm + 1) * P], in_=hstT[:])
    nc.compile()
    return nc


def _build_launch_b(cfg, S, Qb, C, Lq, mode="full"):
    import concourse.mybir as mybir
    import concourse.tile as tile
    from concourse import bacc

    f32 = mybir.dt.float32
    i16 = mybir.dt.int16
    dout, nw, nlp, nq = cfg.dout, cfg.nw, cfg.nlp, cfg.nq
    nr = cfg.m * nlp
    nc = bacc.Bacc("TRN2", target_bir_lowering=False, debug=False,
                   enable_asserts=False, num_devices=cfg.m)
    hsf = nc.dram_tensor("hsf", [nr, dout], f32, kind="ExternalInput")
    hsTo = nc.dram_tensor("hsT", [dout, nlp], f32, kind="ExternalInput")
    dinvT = nc.dram_tensor("dinvT", [dout, nlp], f32, kind="ExternalInput")
    bcol = nc.dram_tensor("bcol", [dout, 1], f32, kind="ExternalInput")
    iot = nc.dram_tensor("iota", [P, P], f32, kind="ExternalInput")
    dsh = nc.dram_tensor("dsh", [P, max(C, 1)], f32, kind="ExternalInput")
    idxq = [nc.dram_tensor(f"idx{q}", [P, int(Lq[q]) // 16], i16, kind="ExternalInput")
            if Lq[q] else None for q in range(nq)]
    outT = nc.dram_tensor("outT", [dout, nlp], f32, kind="ExternalOutput")
    AT = mybir.AluOpType
    with tile.TileContext(nc) as tc:
        with tc.tile_pool(name="const", bufs=1) as cpool, \
             tc.tile_pool(name="msg", bufs=2) as mpool, \
             tc.tile_pool(name="ind", bufs=6) as ipool, \
             tc.tile_pool(name="fin", bufs=6) as fpool, \
             tc.tile_pool(name="own", bufs=2) as opool, \
             tc.tile_pool(name="outp", bufs=2) as tpool, \
             tc.tile_pool(name="psum", bufs=4, space="PSUM") as ppool:
            iotsb = cpool.tile([P, P], f32)
            nc.sync.dma_start(out=iotsb[:], in_=iot[:, :])
            bsb = cpool.tile([dout, 1], f32)
            nc.sync.dma_start(out=bsb[:], in_=bcol[:, :])
            dshsb = cpool.tile([P, max(C, 1)], f32)
            nc.sync.dma_start(out=dshsb[:], in_=dsh[:, :])
            idxsb = []
            for q in range(nq):
                if Lq[q]:
                    t = cpool.tile([P, int(Lq[q]) // 16], i16, tag=f"idx{q}")
                    nc.sync.dma_start(out=t[:], in_=idxq[q][:, :])
                    idxsb.append(t)
                else:
                    idxsb.append(None)
            for sb, ws in enumerate(cfg.sbs):
                w0 = ws[0]
                nwsb = len(ws)
                ownT = opool.tile([dout, nwsb * P], f32, tag="own")
                nc.sync.dma_start(out=ownT[:], in_=hsTo[:, w0 * P:(w0 + nwsb) * P])
                dvT = opool.tile([dout, nwsb * P], f32, tag="dvT")
                nc.sync.dma_start(out=dvT[:], in_=dinvT[:, w0 * P:(w0 + nwsb) * P])
                msgs = {}
                for q in range(nq):
                    nch = int(sum(S[q][w] for w in ws))
                    if nch == 0:
                        continue
                    off = int(sum(S[q][w] for w in range(w0)))
                    mt = mpool.tile([P, nch * dout], f32, tag=f"msg{q}")
                    qs = q * BUCKET
                    qe = min(nr, (q + 1) * BUCKET)
                    MAXCH = 32  # <=64 chunks/call (single-packet+ring limits)
                    for c0 in range(0, nch, MAXCH):
                        c1 = min(c0 + MAXCH, nch)
                        nc.gpsimd.dma_gather(
                            out_ap=mt[:].rearrange("p (c e) -> p c e", e=dout)[:, c0:c1, :],
                            in_ap=hsf[qs:qe, :],
                            idxs_ap=idxsb[q][:, (off + c0) * 8:(off + c1) * 8],
                            num_idxs=(c1 - c0) * P,
                            num_idxs_reg=(c1 - c0) * P,
                            elem_size=dout,
                            single_packet=False,
                        )
                    msgs[q] = (mt, off)
                out_t = tpool.tile([dout, nwsb * P], f32, tag="o")
                if mode == "gather_only":
                    for q, (mt, off) in msgs.items():
                        nc.vector.tensor_copy(out=out_t[:, 0:P], in_=mt[:64, 0:P])
                    nc.sync.dma_start(out=outT[:, w0 * P:(w0 + nwsb) * P], in_=out_t[:])
                    continue
                for wi, w in enumerate(ws):
                    nch_w = int(sum(S[q][w] for q in range(nq)))
                    ci = 0
                    ps = None
                    if nch_w:
                        ps = ppool.tile([dout, P], f32, tag="ps")
                        for q in range(nq):
                            if S[q][w] == 0:
                                continue
                            mt, off = msgs[q]
                            lo = int(sum(S[q][w2] for w2 in ws[:wi]))
                            g0 = int(Qb[q]) + off + lo
                            for i in range(int(S[q][w])):
                                ind = ipool.tile([P, P], f32, tag="ind")
                                nc.vector.tensor_tensor(
                                    out=ind[:],
                                    in0=dshsb[:, g0 + i:g0 + i + 1].to_broadcast([P, P]),
                                    in1=iotsb[:],
                                    op=AT.is_equal,
                                )
                                nc.tensor.matmul(
                                    out=ps[:],
                                    lhsT=mt[:, (lo + i) * dout:(lo + i + 1) * dout],
                                    rhs=ind[:],
                                    start=(ci == 0),
                                    stop=(ci == nch_w - 1),
                                )
                                ci += 1
                        if mode == "no_final":
                            nc.vector.tensor_copy(out=out_t[:, wi * P:(wi + 1) * P], in_=ps[:])
                            continue
                        t1 = fpool.tile([dout, P], f32, tag="t1")
                        nc.vector.tensor_tensor(out=t1[:], in0=ps[:],
                                                in1=ownT[:, wi * P:(wi + 1) * P], op=AT.add)
                        t1ap = t1[:]
                    else:
                        if mode == "no_final":
                            nc.vector.tensor_copy(out=out_t[:, wi * P:(wi + 1) * P],
                                                  in_=ownT[:, wi * P:(wi + 1) * P])
                            continue
                        t1ap = ownT[:, wi * P:(wi + 1) * P]
                    t2 = fpool.tile([dout, P], f32, tag="t2")
                    nc.vector.tensor_tensor(out=t2[:], in0=t1ap,
                                            in1=dvT[:, wi * P:(wi + 1) * P], op=AT.mult)
                    nc.scalar.activation(out=out_t[:, wi * P:(wi + 1) * P], in_=t2[:],
                                         func=mybir.ActivationFunctionType.Relu,
                                         bias=bsb[:, 0:1])
                nc.sync.dma_start(out=outT[:, w0 * P:(w0 + nwsb) * P], in_=out_t[:])
    nc.compile()
    return nc


def _get_kernels(cfg, S, Qb, C, Lq):
    key = (cfg.n, cfg.din, cfg.dout, cfg.m, S.tobytes())
    if key not in _cache:
        _cache[key] = (_build_launch_a(cfg), _build_launch_b(cfg, S, Qb, C, Lq))
    return _cache[key]


def run(cfg, x, edge_index, W, b, trace=False):
    from concourse import bass_utils

    x = np.ascontiguousarray(np.asarray(x, np.float32))
    W = np.ascontiguousarray(np.asarray(W, np.float32))
    b = np.ascontiguousarray(np.asarray(b, np.float32))
    nl, nlp, nw, nq, m, dout = cfg.nl, cfg.nlp, cfg.nw, cfg.nq, cfg.m, cfg.dout

    S, Qb, C, Lq, percore = _preprocess(cfg, edge_index)
    nca, ncb = _get_kernels(cfg, S, Qb, C, Lq)

    ident = np.eye(P, dtype=np.float32)
    in_maps_a = []
    for k in range(m):
        xp = np.zeros((nlp, cfg.din), np.float32)
        xp[:nl] = x[k * nl:(k + 1) * nl]
        in_maps_a.append({
            "xT": np.ascontiguousarray(xp.T),
            "W": W,
            "cnt": percore[k]["cnt2d"],
            "ident": ident,
        })
    import time as _time
    _t0 = _time.time()
    res_a = bass_utils.run_bass_kernel_spmd(nca, in_maps_a, core_ids=list(range(m)),
                                            trace=trace)
    _wall_a = _time.time() - _t0
    hs_full = np.concatenate([res_a.results[k]["hs"] for k in range(m)], axis=0)

    iota = np.tile(np.arange(P, dtype=np.float32), (P, 1))
    in_maps_b = []
    for k in range(m):
        dinv2d = res_a.results[k]["dinv"]  # [P, nw]
        dinv1d = np.ascontiguousarray(dinv2d.T).reshape(nlp)
        in_map = {
            "hsf": hs_full,
            "hsT": res_a.results[k]["hsT"],
            "dinvT": np.ascontiguousarray(np.broadcast_to(dinv1d, (dout, nlp))),
            "bcol": np.ascontiguousarray(b.reshape(dout, 1)),
            "iota": iota,
            "dsh": percore[k]["dsh"] if C else np.zeros((P, 1), np.float32),
        }
        for q in range(nq):
            if Lq[q]:
                in_map[f"idx{q}"] = percore[k]["idxs"][q]
        in_maps_b.append(in_map)
    _t0 = _time.time()
    res_b = bass_utils.run_bass_kernel_spmd(ncb, in_maps_b, core_ids=list(range(m)),
                                            trace=trace)
    _wall_b = _time.time() - _t0
    out = np.concatenate(
        [np.ascontiguousarray(res_b.results[k]["outT"].T)[:nl] for k in range(m)],
        axis=0)
    times = (res_a.exec_time_ns, res_b.exec_time_ns)
    if times[0] is None:
        times = (int(_wall_a * 1e9), int(_wall_b * 1e9))
    return out, times


def kernel(x, edge_index, W, b):
    cfg = GCNConfig()
    out, _ = run(cfg, x, edge_index, W, b)
    return out.astype(np.float32)



# revision 2
# speedup vs baseline: 7.5542x; 7.5542x over previous
"""GCNConv (normalize=True, self-loops) + ReLU on 8 Trainium2 NeuronCores.

Single fused launch (1D node partition, per sharding hint):
  - nodes sharded 8 ways; core k owns rows [k*12500, (k+1)*12500) and all
    edges whose DESTINATION is local.
  - phase A (per core): h = x_k @ W on PE (bf16 in, f32 acc),
    dinv = 1/sqrt(deg+1), hs = h*dinv kept in SBUF + one DMA to a DRAM
    bounce tile.
  - on-device AllGather (gpsimd collective_compute over NeuronLink) of the
    per-core hs shards into one [8*nlp, 64] table — no host round trip.
  - phase B (per core): for each 128-dest window, gather source rows of hs
    (dma_gather, int16 indices per 32768-row bucket), build 0/1 dest
    indicator per 128-edge chunk on DVE (is_equal vs iota), and segment-sum
    via PE matmul accumulating in PSUM [128 dest x 64 feat]; finally
    (+hs_own) * dinv + b, relu -> bf16 output.

Byte-lean transfers (the axon tunnel is the bottleneck, ~25 MB/s):
  x ships bf16; gather indices ship compact [16, L/16] int16 and are
  replicated 8x on device; dsh ships bf16; output ships bf16.

Edges are bucketed by (source-bucket q, dest-window w) with a chunk schedule
S[q][w] shared across cores (max over cores) so all 8 cores run one NEFF.
"""
import sys

sys.path.insert(0, "/opt/trn_rl_repo")
import numpy as np
import ml_dtypes

BF16 = ml_dtypes.bfloat16

N = 100000
DIN = 256
DOUT = 64
M = 8
P = 128
BUCKET = 32768

_cache = {}


def _ceil_div(a, b):
    return (a + b - 1) // b


class GCNConfig:
    def __init__(self, n=N, din=DIN, dout=DOUT, m=M, sbw=7):
        self.n = n
        self.din = din
        self.dout = dout
        self.m = m
        self.nl = n // m
        assert self.nl * m == n
        self.nw = _ceil_div(self.nl, P)
        self.nlp = self.nw * P
        self.nq = _ceil_div(m * self.nlp, BUCKET)
        self.sbw = sbw
        self.sbs = [range(i, min(i + sbw, self.nw)) for i in range(0, self.nw, sbw)]


def _preprocess(cfg, edge_index):
    """Partition + bucket edges; build per-core gather streams and the shared
    chunk schedule. Returns (S, Qb, C, Lq, percore_arrays)."""
    nl, nw, nlp, nq, m = cfg.nl, cfg.nw, cfg.nlp, cfg.nq, cfg.m
    ei = np.asarray(edge_index, dtype=np.int64)
    row, col = ei[0], ei[1]
    kown = col // nl
    dl = col % nl
    gsrc = (row // nl) * nlp + (row % nl)
    qb_ = gsrc // BUCKET

    cores = []
    cnts = np.zeros((m, nq, nw), np.int64)
    for k in range(m):
        sel = kown == k
        dlk = dl[sel]
        gk = gsrc[sel]
        qk = qb_[sel]
        o = np.lexsort((dlk, qk))
        dlk, gk, qk = dlk[o], gk[o], qk[o]
        wk = dlk // P
        cnts[k] = np.bincount(qk * nw + wk, minlength=nq * nw).reshape(nq, nw)
        cores.append((dlk, gk, qk, wk))

    S = _ceil_div(cnts.max(axis=0), P)  # [nq, nw] chunks per group
    Sq = S.sum(axis=1)  # chunks per stream q
    Lq = Sq * P  # idx slots per stream q
    Qb = np.concatenate([[0], np.cumsum(Sq)])  # global chunk base per q
    C = int(Qb[-1])
    chb = np.cumsum(S, axis=1) - S  # chunk base of (q,w) within stream q

    percore = []
    for k in range(m):
        dlk, gk, qk, wk = cores[k]
        nk = len(dlk)
        key = qk * nw + wk
        if nk:
            starts = np.r_[0, np.flatnonzero(np.diff(key)) + 1]
            lens = np.diff(np.r_[starts, nk])
            j = np.arange(nk) - np.repeat(starts, lens)
        else:
            j = np.zeros(0, np.int64)
        pos = chb[qk, wk] * P + j  # slot within stream q
        gpos = (Qb[qk] + chb[qk, wk]) * P + j  # global slot
        idxs = []
        for q in range(nq):
            arr = np.zeros(int(Lq[q]), np.int16)
            selq = qk == q
            arr[pos[selq]] = (gk[selq] % BUCKET).astype(np.int16)
            if Lq[q]:
                a = np.ascontiguousarray(arr.reshape(-1, 16).T)  # [16, Lq/16]
            else:
                a = np.zeros((16, 0), np.int16)
            idxs.append(a)
        dshT = np.full(C * P, -1.0, np.float32)
        dshT[gpos] = (dlk - wk * P).astype(np.float32)
        dsh = np.ascontiguousarray(dshT.reshape(C, P).T).astype(BF16)
        cnt2d = np.ascontiguousarray(
            np.bincount(dlk, minlength=nlp).reshape(nw, P).T
        ).astype(np.float32)
        percore.append({"idxs": idxs, "dsh": dsh, "cnt2d": cnt2d})
    return S, Qb, C, Lq, percore


def _build_kernel(cfg, S, Qb, C, Lq):
    import concourse.mybir as mybir
    import concourse.tile as tile
    from concourse import bacc

    f32 = mybir.dt.float32
    bf16 = mybir.dt.bfloat16
    i16 = mybir.dt.int16
    din, dout, nw, nlp, nq, m = cfg.din, cfg.dout, cfg.nw, cfg.nlp, cfg.nq, cfg.m
    kc = din // P
    nr = m * nlp
    AT = mybir.AluOpType

    nc = bacc.Bacc("TRN2", target_bir_lowering=False, debug=False,
                   enable_asserts=False, num_devices=m)
    xT = nc.dram_tensor("xT", [din, nlp], bf16, kind="ExternalInput")
    Wt = nc.dram_tensor("W", [din, dout], bf16, kind="ExternalInput")
    cnt = nc.dram_tensor("cnt", [P, nw], f32, kind="ExternalInput")
    bb = nc.dram_tensor("bb", [P, dout], f32, kind="ExternalInput")
    iot = nc.dram_tensor("iota", [P, P], bf16, kind="ExternalInput")
    dsh = nc.dram_tensor("dsh", [P, max(C, 1)], bf16, kind="ExternalInput")
    idxq = [nc.dram_tensor(f"idx{q}", [16, int(Lq[q]) // 16], i16,
                           kind="ExternalInput")
            if Lq[q] else None for q in range(nq)]
    outm = nc.dram_tensor("out", [nlp, dout], bf16, kind="ExternalOutput")

    with tile.TileContext(nc) as tc:
        with tc.tile_pool(name="const", bufs=1) as cpool, \
             tc.tile_pool(name="dram", bufs=1, space="DRAM") as dram, \
             tc.tile_pool(name="psum", bufs=4, space="PSUM") as ppool:
            iotsb = cpool.tile([P, P], bf16)
            nc.sync.dma_start(out=iotsb[:], in_=iot[:, :])
            bbsb = cpool.tile([P, dout], f32)
            nc.sync.dma_start(out=bbsb[:], in_=bb[:, :])
            dshsb = cpool.tile([P, max(C, 1)], bf16)
            nc.sync.dma_start(out=dshsb[:], in_=dsh[:, :])
            idxsb = []
            for q in range(nq):
                if Lq[q]:
                    t = cpool.tile([P, int(Lq[q]) // 16], i16, tag=f"idx{q}")
                    for r in range(8):
                        nc.sync.dma_start(out=t[16 * r:16 * (r + 1), :],
                                          in_=idxq[q][:, :])
                    idxsb.append(t)
                else:
                    idxsb.append(None)
            cntsb = cpool.tile([P, nw], f32)
            nc.sync.dma_start(out=cntsb[:], in_=cnt[:, :])
            ssb = cpool.tile([P, nw], f32)
            nc.scalar.activation(out=ssb[:], in_=cntsb[:],
                                 func=mybir.ActivationFunctionType.Sqrt, bias=1.0)
            dsb = cpool.tile([P, nw], f32)
            nc.vector.reciprocal(out=dsb[:], in_=ssb[:])

            # hs kept resident in SBUF ([P, nw, dout]); row w*P+p <-> [p, w, :]
            hssb = cpool.tile([P, nw, dout], f32)
            hs_loc = dram.tile([nlp, dout], f32)
            hs_all = dram.tile([nr, dout], f32)

            # ---- phase A: h = x @ W, hs = h * dinv ----
            with tc.tile_pool(name="xa", bufs=1) as apool:
                xsb = apool.tile([P, kc, nlp], bf16)
                nc.sync.dma_start(
                    out=xsb[:], in_=xT[:, :].rearrange("(c p) m -> p c m", p=P))
                wsb = apool.tile([P, kc, dout], bf16)
                nc.sync.dma_start(
                    out=wsb[:], in_=Wt[:, :].rearrange("(c p) n -> p c n", p=P))
                for mm in range(nw):
                    ps = ppool.tile([P, dout], f32, tag="mma")
                    for c in range(kc):
                        nc.tensor.matmul(out=ps[:],
                                         lhsT=xsb[:, c, mm * P:(mm + 1) * P],
                                         rhs=wsb[:, c, :],
                                         start=(c == 0), stop=(c == kc - 1))
                    nc.vector.tensor_scalar_mul(out=hssb[:, mm, :], in0=ps[:],
                                                scalar1=dsb[:, mm:mm + 1])
            nc.gpsimd.dma_start(
                out=hs_loc.rearrange("(w p) f -> p w f", p=P), in_=hssb[:])

            # ---- all-gather hs shards over NeuronLink ----
            nc.gpsimd.collective_compute(
                "AllGather",
                AT.bypass,
                replica_groups=[list(range(m))],
                ins=[hs_loc.opt()],
                outs=[hs_all.opt()],
            )

            # ---- phase B: gather + segment-sum + finalize ----
            with tc.tile_pool(name="msg", bufs=2) as mpool, \
                 tc.tile_pool(name="ind", bufs=6) as ipool, \
                 tc.tile_pool(name="fin", bufs=6) as fpool, \
                 tc.tile_pool(name="outp", bufs=2) as tpool:
                for sb, ws in enumerate(cfg.sbs):
                    w0 = ws[0]
                    nwsb = len(ws)
                    msgs = {}
                    for q in range(nq):
                        nch = int(sum(S[q][w] for w in ws))
                        if nch == 0:
                            continue
                        off = int(sum(S[q][w] for w in range(w0)))
                        mt = mpool.tile([P, nch, dout], f32, tag=f"msg{q}")
                        qs = q * BUCKET
                        qe = min(nr, (q + 1) * BUCKET)
                        MAXCH = 32  # <=64 chunks/call (single-packet+ring limits)
                        for c0 in range(0, nch, MAXCH):
                            c1 = min(c0 + MAXCH, nch)
                            nc.gpsimd.dma_gather(
                                out_ap=mt[:, c0:c1, :],
                                in_ap=hs_all[qs:qe, :],
                                idxs_ap=idxsb[q][:, (off + c0) * 8:(off + c1) * 8],
                                num_idxs=(c1 - c0) * P,
                                num_idxs_reg=(c1 - c0) * P,
                                elem_size=dout,
                                single_packet=False,
                            )
                        msgs[q] = (mt, off)
                    out_t = tpool.tile([P, nwsb, dout], bf16, tag="o")
                    for wi, w in enumerate(ws):
                        nch_w = int(sum(S[q][w] for q in range(nq)))
                        own = hssb[:, w, :]
                        if nch_w:
                            ci = 0
                            ps = ppool.tile([P, dout], f32, tag="psb")
                            for q in range(nq):
                                if S[q][w] == 0:
                                    continue
                                mt, off = msgs[q]
                                lo = int(sum(S[q][w2] for w2 in ws[:wi]))
                                g0 = int(Qb[q]) + off + lo
                                for i in range(int(S[q][w])):
                                    ind = ipool.tile([P, P], f32, tag="ind")
                                    nc.vector.tensor_tensor(
                                        out=ind[:],
                                        in0=dshsb[:, g0 + i:g0 + i + 1].to_broadcast([P, P]),
                                        in1=iotsb[:],
                                        op=AT.is_equal,
                                    )
                                    nc.tensor.matmul(
                                        out=ps[:],
                                        lhsT=ind[:],
                                        rhs=mt[:, lo + i, :],
                                        start=(ci == 0),
                                        stop=(ci == nch_w - 1),
                                    )
                                    ci += 1
                            t1 = fpool.tile([P, dout], f32, tag="t1")
                            nc.vector.tensor_tensor(out=t1[:], in0=ps[:], in1=own,
                                                    op=AT.add)
                            t1ap = t1[:]
                        else:
                            t1ap = own
                        t2 = fpool.tile([P, dout], f32, tag="t2")
                        nc.vector.tensor_scalar_mul(out=t2[:], in0=t1ap,
                                                    scalar1=dsb[:, w:w + 1])
                        t3 = fpool.tile([P, dout], f32, tag="t3")
                        nc.vector.tensor_tensor(out=t3[:], in0=t2[:], in1=bbsb[:],
                                                op=AT.add)
                        nc.scalar.activation(out=out_t[:, wi, :], in_=t3[:],
                                             func=mybir.ActivationFunctionType.Relu,
                                             bias=0.0)
                    nc.sync.dma_start(
                        out=outm[w0 * P:(w0 + nwsb) * P, :].rearrange(
                            "(w p) f -> p w f", p=P),
                        in_=out_t[:])
    nc.compile()
    return nc


def _get_kernel(cfg, S, Qb, C, Lq):
    key = (cfg.n, cfg.din, cfg.dout, cfg.m, S.tobytes())
    if key not in _cache:
        _cache[key] = _build_kernel(cfg, S, Qb, C, Lq)
    return _cache[key]


def run(cfg, x, edge_index, W, b, trace=False):
    from concourse import bass_utils

    x = np.asarray(x, np.float32)
    W = np.asarray(W, np.float32)
    b = np.asarray(b, np.float32)
    nl, nlp, nw, nq, m, dout = cfg.nl, cfg.nlp, cfg.nw, cfg.nq, cfg.m, cfg.dout

    S, Qb, C, Lq, percore = _preprocess(cfg, edge_index)
    nck = _get_kernel(cfg, S, Qb, C, Lq)

    xbf = x.astype(BF16)
    Wbf = np.ascontiguousarray(W.astype(BF16))
    bbc = np.ascontiguousarray(
        np.broadcast_to(b.astype(np.float32), (P, dout)))
    iota = np.tile(np.arange(P, dtype=np.float32), (P, 1)).astype(BF16)
    in_maps = []
    for k in range(m):
        xp = np.zeros((nlp, cfg.din), BF16)
        xp[:nl] = xbf[k * nl:(k + 1) * nl]
        in_map = {
            "xT": np.ascontiguousarray(xp.T),
            "W": Wbf,
            "cnt": percore[k]["cnt2d"],
            "bb": bbc,
            "iota": iota,
            "dsh": percore[k]["dsh"] if C else np.full((P, 1), -1, BF16),
        }
        for q in range(nq):
            if Lq[q]:
                in_map[f"idx{q}"] = percore[k]["idxs"][q]
        in_maps.append(in_map)
    import time as _time
    _t0 = _time.time()
    res = bass_utils.run_bass_kernel_spmd(nck, in_maps, core_ids=list(range(m)),
                                          trace=trace)
    _wall = _time.time() - _t0
    out = np.concatenate(
        [np.asarray(res.results[k]["out"]).astype(np.float32)[:nl]
         for k in range(m)], axis=0)
    t = res.exec_time_ns
    if t is None:
        t = int(_wall * 1e9)
    return out, (t,)


def kernel(x, edge_index, W, b):
    cfg = GCNConfig()
    out, _ = run(cfg, x, edge_index, W, b)
    return out.astype(np.float32)


# revision 8
# speedup vs baseline: 11.2168x; 1.4848x over previous
"""GCNConv (normalize=True, self-loops) + ReLU on 8 Trainium2 NeuronCores.

Single fused launch (1D node partition, per sharding hint):
  - nodes sharded 8 ways; core k owns rows [k*12500, (k+1)*12500) and all
    edges whose DESTINATION is local.
  - phase A (per core): h = x_k @ W on PE (bf16 in, f32 acc),
    dinv = 1/sqrt(deg+1), hs = h*dinv kept in SBUF + one DMA to a DRAM
    bounce tile.
  - on-device AllGather (gpsimd collective_compute over NeuronLink) of the
    per-core hs shards into one [8*nlp, 64] table — no host round trip.
  - phase B (per core): for each 128-dest window, gather source rows of hs
    (dma_gather, int16 indices per 32768-row bucket), build 0/1 dest
    indicator per 128-edge chunk on DVE (is_equal vs iota), and segment-sum
    via PE matmul accumulating in PSUM [128 dest x 64 feat]; finally
    (+hs_own) * dinv + b, relu -> bf16 output.

Byte-lean transfers (the axon tunnel is the bottleneck, ~35-70 MB/s):
  x ships bf16; gather indices ship compact [16, L/16] int16 and are
  replicated 8x on device; dsh ships int8; iota is generated on device;
  output ships bf16.

Edges are bucketed by (source-bucket q, dest-window w) with a chunk schedule
S[q][w] shared across cores (max over cores) so all 8 cores run one NEFF.
"""
import sys

sys.path.insert(0, "/opt/trn_rl_repo")
import numpy as np
import ml_dtypes

BF16 = ml_dtypes.bfloat16

N = 100000
DIN = 256
DOUT = 64
M = 8
P = 128
BUCKET = 32768

_cache = {}


def _ceil_div(a, b):
    return (a + b - 1) // b


class GCNConfig:
    def __init__(self, n=N, din=DIN, dout=DOUT, m=M, sbw=7):
        self.n = n
        self.din = din
        self.dout = dout
        self.m = m
        self.nl = n // m
        assert self.nl * m == n
        self.nw = _ceil_div(self.nl, P)
        self.nlp = self.nw * P
        self.nq = _ceil_div(m * self.nlp, BUCKET)
        self.sbw = sbw
        self.sbs = [range(i, min(i + sbw, self.nw)) for i in range(0, self.nw, sbw)]


def _preprocess(cfg, edge_index):
    """Partition + bucket edges; build per-core gather streams and the shared
    chunk schedule. Returns (S, Qb, C, Lq, percore_arrays)."""
    nl, nw, nlp, nq, m = cfg.nl, cfg.nw, cfg.nlp, cfg.nq, cfg.m
    ei = np.asarray(edge_index, dtype=np.int64)
    row, col = ei[0], ei[1]
    kown = col // nl
    dl = col % nl
    gsrc = (row // nl) * nlp + (row % nl)
    qb_ = gsrc // BUCKET

    cores = []
    cnts = np.zeros((m, nq, nw), np.int64)
    for k in range(m):
        sel = kown == k
        dlk = dl[sel]
        gk = gsrc[sel]
        qk = qb_[sel]
        o = np.lexsort((dlk, qk))
        dlk, gk, qk = dlk[o], gk[o], qk[o]
        wk = dlk // P
        cnts[k] = np.bincount(qk * nw + wk, minlength=nq * nw).reshape(nq, nw)
        cores.append((dlk, gk, qk, wk))

    S = _ceil_div(cnts.max(axis=0), P)  # [nq, nw] chunks per group
    Sq = S.sum(axis=1)  # chunks per stream q
    Lq = Sq * P  # idx slots per stream q
    Qb = np.concatenate([[0], np.cumsum(Sq)])  # global chunk base per q
    C = int(Qb[-1])
    chb = np.cumsum(S, axis=1) - S  # chunk base of (q,w) within stream q

    percore = []
    for k in range(m):
        dlk, gk, qk, wk = cores[k]
        nk = len(dlk)
        key = qk * nw + wk
        if nk:
            starts = np.r_[0, np.flatnonzero(np.diff(key)) + 1]
            lens = np.diff(np.r_[starts, nk])
            j = np.arange(nk) - np.repeat(starts, lens)
        else:
            j = np.zeros(0, np.int64)
        pos = chb[qk, wk] * P + j  # slot within stream q
        gpos = (Qb[qk] + chb[qk, wk]) * P + j  # global slot
        idxs = []
        for q in range(nq):
            arr = np.zeros(int(Lq[q]), np.int16)
            selq = qk == q
            arr[pos[selq]] = (gk[selq] % BUCKET).astype(np.int16)
            if Lq[q]:
                a = np.ascontiguousarray(arr.reshape(-1, 16).T)  # [16, Lq/16]
            else:
                a = np.zeros((16, 0), np.int16)
            idxs.append(a)
        dshT = np.full(C * P, -1.0, np.float32)
        dshT[gpos] = (dlk - wk * P).astype(np.float32)
        dsh = np.ascontiguousarray(dshT.reshape(C, P).T).astype(np.int8)
        cnt2d = np.ascontiguousarray(
            np.bincount(dlk, minlength=nlp).reshape(nw, P).T
        ).astype(np.float32)
        percore.append({"idxs": idxs, "dsh": dsh, "cnt2d": cnt2d})
    return S, Qb, C, Lq, percore


def _build_kernel(cfg, S, Qb, C, Lq, mode="full"):
    import concourse.mybir as mybir
    import concourse.tile as tile
    from concourse import bacc

    f32 = mybir.dt.float32
    bf16 = mybir.dt.bfloat16
    i16 = mybir.dt.int16
    din, dout, nw, nlp, nq, m = cfg.din, cfg.dout, cfg.nw, cfg.nlp, cfg.nq, cfg.m
    kc = din // P
    nr = m * nlp
    AT = mybir.AluOpType

    nc = bacc.Bacc("TRN2", target_bir_lowering=False, debug=False,
                   enable_asserts=False, num_devices=m)
    xT = nc.dram_tensor("xT", [din, nlp], bf16, kind="ExternalInput")
    Wt = nc.dram_tensor("W", [din, dout], bf16, kind="ExternalInput")
    cnt = nc.dram_tensor("cnt", [P, nw], f32, kind="ExternalInput")
    bb = nc.dram_tensor("bb", [P, dout], f32, kind="ExternalInput")
    i8 = mybir.dt.int8
    dsh = nc.dram_tensor("dsh", [P, max(C, 1)], i8, kind="ExternalInput")
    idxq = [nc.dram_tensor(f"idx{q}", [16, int(Lq[q]) // 16], i16,
                           kind="ExternalInput")
            if Lq[q] else None for q in range(nq)]
    outm = nc.dram_tensor("out", [nlp, dout], bf16, kind="ExternalOutput")

    with tile.TileContext(nc) as tc:
        with tc.tile_pool(name="const", bufs=1) as cpool, \
             tc.tile_pool(name="dram", bufs=1, space="DRAM") as dram, \
             tc.tile_pool(name="psum", bufs=4, space="PSUM") as ppool:
            iotsb = cpool.tile([P, P], f32)
            nc.gpsimd.iota(iotsb[:], [[1, P]], channel_multiplier=0,
                           allow_small_or_imprecise_dtypes=True)
            bbsb = cpool.tile([P, dout], f32)
            nc.sync.dma_start(out=bbsb[:], in_=bb[:, :])
            dsh8 = cpool.tile([P, max(C, 1)], i8)
            nc.sync.dma_start(out=dsh8[:], in_=dsh[:, :])
            dshsb = cpool.tile([P, max(C, 1)], f32)
            nc.vector.tensor_copy(out=dshsb[:], in_=dsh8[:])
            idxsb = []
            for q in range(nq):
                if Lq[q]:
                    t = cpool.tile([P, int(Lq[q]) // 16], i16, tag=f"idx{q}")
                    for r in range(8):
                        nc.sync.dma_start(out=t[16 * r:16 * (r + 1), :],
                                          in_=idxq[q][:, :])
                    idxsb.append(t)
                else:
                    idxsb.append(None)
            cntsb = cpool.tile([P, nw], f32)
            nc.sync.dma_start(out=cntsb[:], in_=cnt[:, :])
            ssb = cpool.tile([P, nw], f32)
            nc.scalar.activation(out=ssb[:], in_=cntsb[:],
                                 func=mybir.ActivationFunctionType.Sqrt, bias=1.0)
            dsb = cpool.tile([P, nw], f32)
            nc.vector.reciprocal(out=dsb[:], in_=ssb[:])

            # hs kept resident in SBUF ([P, nw, dout]); row w*P+p <-> [p, w, :]
            hssb = cpool.tile([P, nw, dout], f32)
            hs_loc = dram.tile([nlp, dout], f32)
            hs_all = dram.tile([nr, dout], f32)

            # ---- phase A: h = x @ W, hs = h * dinv ----
            with tc.tile_pool(name="xa", bufs=1) as apool:
                xsb = apool.tile([P, kc, nlp], bf16)
                nc.sync.dma_start(
                    out=xsb[:], in_=xT[:, :].rearrange("(c p) m -> p c m", p=P))
                wsb = apool.tile([P, kc, dout], bf16)
                nc.sync.dma_start(
                    out=wsb[:], in_=Wt[:, :].rearrange("(c p) n -> p c n", p=P))
                for mm in range(nw):
                    ps = ppool.tile([P, dout], f32, tag="mma")
                    for c in range(kc):
                        nc.tensor.matmul(out=ps[:],
                                         lhsT=xsb[:, c, mm * P:(mm + 1) * P],
                                         rhs=wsb[:, c, :],
                                         start=(c == 0), stop=(c == kc - 1))
                    nc.vector.tensor_scalar_mul(out=hssb[:, mm, :], in0=ps[:],
                                                scalar1=dsb[:, mm:mm + 1])
            nc.gpsimd.dma_start(
                out=hs_loc.rearrange("(w p) f -> p w f", p=P), in_=hssb[:])

            # ---- all-gather hs shards over NeuronLink ----
            if mode != "no_collective":
                nc.gpsimd.collective_compute(
                    "AllGather",
                    AT.bypass,
                    replica_groups=[list(range(m))],
                    ins=[hs_loc.opt()],
                    outs=[hs_all.opt()],
                )

            # ---- phase B: gather + segment-sum + finalize ----
            with tc.tile_pool(name="msg", bufs=2) as mpool, \
                 tc.tile_pool(name="ind", bufs=6) as ipool, \
                 tc.tile_pool(name="fin", bufs=6) as fpool, \
                 tc.tile_pool(name="outp", bufs=2) as tpool:
                for sb, ws in enumerate(cfg.sbs):
                    w0 = ws[0]
                    nwsb = len(ws)
                    msgs = {}
                    for q in range(nq):
                        if mode == "a_only":
                            continue
                        nch = int(sum(S[q][w] for w in ws))
                        if nch == 0:
                            continue
                        off = int(sum(S[q][w] for w in range(w0)))
                        mt = mpool.tile([P, nch, dout], f32, tag=f"msg{q}")
                        qs = q * BUCKET
                        qe = min(nr, (q + 1) * BUCKET)
                        MAXCH = 32  # <=64 chunks/call (single-packet+ring limits)
                        for c0 in range(0, nch, MAXCH):
                            c1 = min(c0 + MAXCH, nch)
                            nc.gpsimd.dma_gather(
                                out_ap=mt[:, c0:c1, :],
                                in_ap=hs_all[qs:qe, :],
                                idxs_ap=idxsb[q][:, (off + c0) * 8:(off + c1) * 8],
                                num_idxs=(c1 - c0) * P,
                                num_idxs_reg=(c1 - c0) * P,
                                elem_size=dout,
                                single_packet=False,
                            )
                        msgs[q] = (mt, off)
                    out_t = tpool.tile([P, nwsb, dout], bf16, tag="o")
                    for wi, w in enumerate(ws):
                        nch_w = 0 if mode == "a_only" else int(
                            sum(S[q][w] for q in range(nq)))
                        own = hssb[:, w, :]
                        if nch_w:
                            ci = 0
                            ps = ppool.tile([P, dout], f32, tag="psb")
                            for q in range(nq):
                                if S[q][w] == 0:
                                    continue
                                mt, off = msgs[q]
                                lo = int(sum(S[q][w2] for w2 in ws[:wi]))
                                g0 = int(Qb[q]) + off + lo
                                for i in range(int(S[q][w])):
                                    ind = ipool.tile([P, P], f32, tag="ind")
                                    nc.vector.tensor_tensor(
                                        out=ind[:],
                                        in0=dshsb[:, g0 + i:g0 + i + 1].to_broadcast([P, P]),
                                        in1=iotsb[:],
                                        op=AT.is_equal,
                                    )
                                    nc.tensor.matmul(
                                        out=ps[:],
                                        lhsT=ind[:],
                                        rhs=mt[:, lo + i, :],
                                        start=(ci == 0),
                                        stop=(ci == nch_w - 1),
                                    )
                                    ci += 1
                            t1 = fpool.tile([P, dout], f32, tag="t1")
                            nc.vector.tensor_tensor(out=t1[:], in0=ps[:], in1=own,
                                                    op=AT.add)
                            t1ap = t1[:]
                        else:
                            t1ap = own
                        t2 = fpool.tile([P, dout], f32, tag="t2")
                        nc.vector.tensor_scalar_mul(out=t2[:], in0=t1ap,
                                                    scalar1=dsb[:, w:w + 1])
                        t3 = fpool.tile([P, dout], f32, tag="t3")
                        nc.vector.tensor_tensor(out=t3[:], in0=t2[:], in1=bbsb[:],
                                                op=AT.add)
                        nc.scalar.activation(out=out_t[:, wi, :], in_=t3[:],
                                             func=mybir.ActivationFunctionType.Relu,
                                             bias=0.0)
                    nc.sync.dma_start(
                        out=outm[w0 * P:(w0 + nwsb) * P, :].rearrange(
                            "(w p) f -> p w f", p=P),
                        in_=out_t[:])
    nc.compile()
    return nc


def _get_kernel(cfg, S, Qb, C, Lq):
    key = (cfg.n, cfg.din, cfg.dout, cfg.m, S.tobytes())
    if key not in _cache:
        _cache[key] = _build_kernel(cfg, S, Qb, C, Lq)
    return _cache[key]


def run(cfg, x, edge_index, W, b, trace=False):
    from concourse import bass_utils

    x = np.asarray(x, np.float32)
    W = np.asarray(W, np.float32)
    b = np.asarray(b, np.float32)
    nl, nlp, nw, nq, m, dout = cfg.nl, cfg.nlp, cfg.nw, cfg.nq, cfg.m, cfg.dout

    S, Qb, C, Lq, percore = _preprocess(cfg, edge_index)
    nck = _get_kernel(cfg, S, Qb, C, Lq)

    xbf = x.astype(BF16)
    Wbf = np.ascontiguousarray(W.astype(BF16))
    bbc = np.ascontiguousarray(
        np.broadcast_to(b.astype(np.float32), (P, dout)))
    in_maps = []
    for k in range(m):
        xp = np.zeros((nlp, cfg.din), BF16)
        xp[:nl] = xbf[k * nl:(k + 1) * nl]
        in_map = {
            "xT": np.ascontiguousarray(xp.T),
            "W": Wbf,
            "cnt": percore[k]["cnt2d"],
            "bb": bbc,
            "dsh": percore[k]["dsh"] if C else np.full((P, 1), -1, np.int8),
        }
        for q in range(nq):
            if Lq[q]:
                in_map[f"idx{q}"] = percore[k]["idxs"][q]
        in_maps.append(in_map)
    import time as _time
    _t0 = _time.time()
    res = bass_utils.run_bass_kernel_spmd(nck, in_maps, core_ids=list(range(m)),
                                          trace=trace)
    _wall = _time.time() - _t0
    out = np.concatenate(
        [np.asarray(res.results[k]["out"]).astype(np.float32)[:nl]
         for k in range(m)], axis=0)
    t = res.exec_time_ns
    if t is None:
        t = int(_wall * 1e9)
    return out, (t,)


def kernel(x, edge_index, W, b):
    cfg = GCNConfig()
    out, _ = run(cfg, x, edge_index, W, b)
    return out.astype(np.float32)
